# revision 22
# baseline (speedup 1.0000x reference)
"""Trainium2 Bass kernel for the CFGKT dense transformer (B=8, L=1024, D=512,
H=8, DFF=2048; 2 self-attn+FFN layers on qa_embed, then 4 layers on q_embed
alternating self-attn and cross-attn-to-y).

Sharding: pure data-parallel — one batch element per NeuronCore, zero
collectives.  Inside each core everything runs on a transposed activation
layout ([D, L], d on partitions) so projections are plain lhsT=W matmuls.

Key algorithmic points (validated vs reference in numpy):
  - kq_same=True and query==key input in every layer, so scores S = Q @ Q^T are
    symmetric.  We compute only upper-triangle-by-block tiles of E = exp(S/8)
    ([k-part, q-free] layout) and use each tile both for the row-softmax
    denominator and as the PV right operand — no transposes anywhere.
  - Softmax without max-subtraction (|S/8| is small), denominators via a ones
    column folded into the V stationary operand (even heads: [V|1], odd heads:
    [1|pad|V] with tile_position=(0,32) so ctx rows land partition-aligned).
  - Strictly-causal layers (mask_k=0): row q=0 fully masked -> reciprocal row
    gets column 0 forced to 0 after the reciprocal, giving exactly 0 output.
  - LayerNorm stats over the partition dim via ones-vector matmuls (sum and
    sum-of-squares), applied as (D*x - musum) * rD with
    rD = 1/sqrt(D^2*(var+eps)); per-column vectors are replicated across
    partitions by DMA broadcast.
  - All matmuls run as float32r (full fp32 data, 1 cycle/row at N>=256).
"""

import numpy as np

B, L, D, H, DFF = 8, 1024, 512, 8, 2048
DK = D // H          # 64
PART = 128
DT = D // PART       # 4 d-tiles
NB = L // PART       # 8 L-blocks of 128
LC = L // 512        # 2 L-chunks of 512
FT = DFF // PART     # 16 ff tiles
NCORES = 8
EPS = 1e-5

# layer configs: (stream, vals, strict, ffn, ffn_idx)
LAYERS = [
    ("y", "self", False, True, 0),
    ("y", "self", False, True, 1),
    ("x", "self", False, False, None),
    ("x", "y", True, True, 2),
    ("x", "self", False, False, None),
    ("x", "y", True, True, 3),
]
NFFN = 4

_CACHE = {}


def _emit(nc, ln_degenerate):
    import concourse.bass as bass
    import concourse.tile as tile
    from concourse import mybir
    from contextlib import ExitStack

    f32 = mybir.dt.float32
    f32r = mybir.dt.float32r
    AF = mybir.ActivationFunctionType
    OP = mybir.AluOpType

    def r(ap):
        return ap

    # ---- DRAM I/O ----
    xT_d = nc.dram_tensor("xT", [D, L], f32r, kind="ExternalInput").ap()
    yT_d = nc.dram_tensor("yT", [D, L], f32r, kind="ExternalInput").ap()
    wk_d = nc.dram_tensor("wk", [6, D, D], f32r, kind="ExternalInput").ap()
    wv_d = nc.dram_tensor("wv", [6, D, D], f32r, kind="ExternalInput").ap()
    wo_d = nc.dram_tensor("wo", [6, D, D], f32r, kind="ExternalInput").ap()
    bk_d = nc.dram_tensor("bk", [6, D], f32, kind="ExternalInput").ap()
    bv_d = nc.dram_tensor("bv", [6, D], f32, kind="ExternalInput").ap()
    bo_d = nc.dram_tensor("bo", [6, D], f32, kind="ExternalInput").ap()
    lnw_d = nc.dram_tensor("lnw", [6, 2, D], f32, kind="ExternalInput").ap()
    lnb_d = nc.dram_tensor("lnb", [6, 2, D], f32, kind="ExternalInput").ap()
    w1_d = nc.dram_tensor("w1", [NFFN, D, DFF], f32r, kind="ExternalInput").ap()
    b1_d = nc.dram_tensor("b1", [NFFN, DFF], f32, kind="ExternalInput").ap()
    w2_d = nc.dram_tensor("w2", [NFFN, DFF, D], f32r, kind="ExternalInput").ap()
    b2_d = nc.dram_tensor("b2", [NFFN, D], f32, kind="ExternalInput").ap()
    out_d = nc.dram_tensor("outT", [D, L], f32r, kind="ExternalOutput").ap()

    # masks embedded in the NEFF: [2, 128, 128]; idx 0 causal (k<=q),
    # idx 1 strict (k<q) — multiplicative 0/1, applied post-exp.
    kk = np.arange(PART)[:, None]
    qq = np.arange(PART)[None, :]
    masks_np = np.stack([(kk <= qq), (kk < qq)]).astype(np.float32)
    masks_d = nc.inline_tensor(masks_np, name="masks").ap()

    with tile.TileContext(nc) as tc, ExitStack() as ctx:
        pers = ctx.enter_context(tc.tile_pool(name="pers", bufs=1))
        wkpool = ctx.enter_context(tc.tile_pool(name="wkpool", bufs=2))
        wpool = ctx.enter_context(tc.tile_pool(name="wpool", bufs=1))

        xT = pers.tile([PART, DT, L], f32r, name="xT_sb")
        yT = pers.tile([PART, DT, L], f32r, name="yT_sb")
        masks = pers.tile([PART, 2, PART], f32, name="masks_sb")
        ones = pers.tile([PART, 1], f32r, name="ones_sb")
        epsc = pers.tile([1, 1], f32, name="eps_sb")
        nc.vector.memset(epsc, float(D) * float(D) * EPS)
        # memset can't write f32r; stage 1.0 in f32 and copy (engine rounds)
        ones_f = pers.tile([PART, 1], f32, name="ones_f_sb")
        nc.vector.memset(ones_f, 1.0)
        ones_row = pers.tile([1, PART], f32r, name="ones_row_sb")
        nc.vector.tensor_copy(ones_row, ones_f[0:1, 0:1].to_broadcast([1, PART]))

        # DRAM scratch used to replicate per-column vectors across partitions
        # (SBUF->DRAM then DRAM->SBUF with a step-0 partition source).
        scr_d = nc.dram_tensor("bscr", [256, 512], f32).ap()
        slot_ctr = [0]

        def dram_bcast(dst_ap, src_ap, nparts):
            s = slot_ctr[0]
            slot_ctr[0] += 1
            nc.sync.dma_start(out=scr_d[s:s + 1, :], in_=src_ap)
            nc.sync.dma_start(
                out=dst_ap, in_=scr_d[s:s + 1, :].to_broadcast([nparts, 512]))

        nc.sync.dma_start(out=xT, in_=xT_d.rearrange("(t p) l -> p t l", p=PART))
        nc.sync.dma_start(out=yT, in_=yT_d.rearrange("(t p) l -> p t l", p=PART))
        nc.sync.dma_start(out=masks, in_=masks_d.rearrange("i p q -> p i q"))
        nc.vector.tensor_copy(ones, ones_f)

        cur = {}

        def layer_norm(lc, li, which, src, dst, lname):
            """LN over the partition dim of the transposed stream.
            src/dst: sbuf [128, DT, L]."""
            psum_st = lc.enter_context(
                tc.tile_pool(name=f"pst_{lname}", bufs=2, space="PSUM"))
            spool = lc.enter_context(tc.tile_pool(name=f"sq_{lname}", bufs=2))
            bpool = lc.enter_context(tc.tile_pool(name=f"lnb_{lname}", bufs=2))
            for c in range(LC):
                cs = slice(c * 512, (c + 1) * 512)
                sq = spool.tile([PART, DT, 512], f32r, tag="sq")
                for k in range(DT):
                    nc.scalar.square(out=sq[:, k, :], in_=src[:, k, cs])
                mu = psum_st.tile([1, 512], f32, tag="mu", bufs=1)
                ssq = psum_st.tile([1, 512], f32, tag="ssq", bufs=1)
                for k in range(DT):
                    nc.tensor.matmul(mu, r(ones), r(src[:, k, cs]),
                                     start=(k == 0), stop=(k == DT - 1))
                for k in range(DT):
                    nc.tensor.matmul(ssq, r(ones), r(sq[:, k, :]),
                                     start=(k == 0), stop=(k == DT - 1))
                m2 = spool.tile([1, 512], f32, tag="m2")
                nc.scalar.square(out=m2, in_=mu[0:1, :])
                vD2 = spool.tile([1, 512], f32, tag="vD2")
                nc.vector.scalar_tensor_tensor(
                    out=vD2, in0=ssq[0:1, :], scalar=float(D), in1=m2,
                    op0=OP.mult, op1=OP.subtract)
                # t = sqrt(vD2 + D^2*eps);  rD = 1/t = rstd/D
                tsq = spool.tile([1, 512], f32r, tag="tsq")
                nc.scalar.activation(out=tsq, in_=vD2, func=AF.Sqrt,
                                     bias=epsc, scale=1.0)
                # replicate mu and tsq across partitions via PE (ones-row
                # outer products into psum); rD_b = 1/tsq via fast approx
                mu_sb = spool.tile([1, 512], f32r, tag="mu_sb")
                nc.scalar.copy(out=mu_sb, in_=mu[0:1, :])
                mu_b = psum_st.tile([PART, 512], f32, tag="mu_b", bufs=2)
                nc.tensor.matmul(mu_b, r(ones_row), r(mu_sb),
                                 start=True, stop=True)
                ts_b = psum_st.tile([PART, 512], f32, tag="ts_b", bufs=2)
                nc.tensor.matmul(ts_b, r(ones_row), r(tsq),
                                 start=True, stop=True)
                rD_b = bpool.tile([PART, 512], f32, tag="rD_b")
                nc.vector.reciprocal_approx_fast(out=rD_b, in_=ts_b)
                for k in range(DT):
                    t1 = spool.tile([PART, 512], f32, tag="t1")
                    nc.vector.scalar_tensor_tensor(
                        out=t1, in0=src[:, k, cs], scalar=float(D), in1=mu_b,
                        op0=OP.mult, op1=OP.subtract)
                    if ln_degenerate:
                        nc.vector.tensor_mul(dst[:, k, cs], t1, rD_b)
                    else:
                        t2 = spool.tile([PART, 512], f32, tag="t2")
                        nc.vector.tensor_mul(t2, t1, rD_b)
                        nc.vector.tensor_scalar(
                            out=dst[:, k, cs], in0=t2,
                            scalar1=cur["lnw"][:, which, li:li + 1],
                            scalar2=cur["lnb"][:, which, li:li + 1],
                            op0=OP.mult, op1=OP.add)

        for li, (strm_name, vals_name, strict, ffn, fi) in enumerate(LAYERS):
            strm = yT if strm_name == "y" else xT
            valsT = yT if vals_name == "y" else strm
            lname = f"l{li}"
            with ExitStack() as lc:
                psum_proj = lc.enter_context(
                    tc.tile_pool(name=f"ppj_{lname}", bufs=2, space="PSUM"))
                gt_pool = lc.enter_context(tc.tile_pool(name=f"gt_{lname}", bufs=1))
                GT = gt_pool.tile([PART, DT, L], f32r, name=f"GT_{lname}")

                wk_sb = wkpool.tile([PART, DT, D], f32r, tag="wk", name=f"wk_{lname}")
                wv_sb = wpool.tile([PART, DT, D], f32r, tag="wv", name=f"wv_{lname}")
                wo_sb = wpool.tile([PART, DT, D], f32r, tag="wo", name=f"wo_{lname}")
                bk_sb = wpool.tile([PART, DT], f32, tag="bk", name=f"bk_{lname}")
                bo_sb = wpool.tile([PART, DT], f32, tag="bo", name=f"bo_{lname}")
                bv_b = wpool.tile([PART, D], f32, tag="bv_b", name=f"bvb_{lname}")
                nc.sync.dma_start(out=wk_sb, in_=wk_d[li].rearrange("(k p) n -> p k n", p=PART))
                nc.sync.dma_start(out=wv_sb, in_=wv_d[li].rearrange("(k p) n -> p k n", p=PART))
                nc.sync.dma_start(out=wo_sb, in_=wo_d[li].rearrange("(k p) n -> p k n", p=PART))
                nc.sync.dma_start(out=bk_sb, in_=bk_d[li].rearrange("(k p) -> p k", p=PART))
                nc.sync.dma_start(out=bo_sb, in_=bo_d[li].rearrange("(k p) -> p k", p=PART))
                nc.sync.dma_start(
                    out=bv_b,
                    in_=bv_d[li].unsqueeze(0).to_broadcast([PART, D]))
                if not ln_degenerate:
                    lnw_sb = wpool.tile([PART, 2, DT], f32, tag="lnw", name=f"lnw_{lname}")
                    lnb_sb = wpool.tile([PART, 2, DT], f32, tag="lnb", name=f"lnb_{lname}")
                    nc.sync.dma_start(out=lnw_sb, in_=lnw_d[li].rearrange("i (k p) -> p i k", p=PART))
                    nc.sync.dma_start(out=lnb_sb, in_=lnb_d[li].rearrange("i (k p) -> p i k", p=PART))
                    cur["lnw"], cur["lnb"] = lnw_sb, lnb_sb

                # ---- attention ----
                with ExitStack() as ac:
                    psum_s = ac.enter_context(
                        tc.tile_pool(name=f"ps_{lname}", bufs=3, space="PSUM"))
                    psum_pv = ac.enter_context(
                        tc.tile_pool(name=f"ppv_{lname}", bufs=2, space="PSUM"))
                    epool = ac.enter_context(tc.tile_pool(name=f"ep_{lname}", bufs=2))
                    vpool = ac.enter_context(tc.tile_pool(name=f"vp_{lname}", bufs=1))
                    cpool = ac.enter_context(tc.tile_pool(name=f"cp_{lname}", bufs=1))
                    rpool = ac.enter_context(tc.tile_pool(name=f"rp_{lname}", bufs=3))

                    # QK projection: GT = Wk^T @ strm + bk  (transposed out)
                    # weight-stationary: both L-chunks consume one LDWEIGHTS
                    for c in range(LC):
                        for m in range(DT):
                            pp = psum_proj.tile([PART, 512], f32, tag="proj",
                                                name=f"pp{c}_{m}")
                            for k in range(DT):
                                nc.tensor.matmul(
                                    pp, r(wk_sb[:, k, m * PART:(m + 1) * PART]),
                                    r(strm[:, k, c * 512:(c + 1) * 512]),
                                    start=(k == 0), stop=(k == DT - 1))
                            nc.vector.tensor_scalar(
                                out=GT[:, m, c * 512:(c + 1) * 512], in0=pp,
                                scalar1=bk_sb[:, m:m + 1], scalar2=None,
                                op0=OP.add)

                    # V projection (natural layout + ones col per head):
                    # V1[:, lb, h, 0:64] = V rows of block lb, head h; col 64 = 1
                    V1 = vpool.tile([PART, NB, H, 65], f32r, name=f"V1_{lname}")
                    for lb in range(NB):
                        pp = psum_proj.tile([PART, 512], f32, tag="proj")
                        for k in range(DT):
                            nc.tensor.matmul(
                                pp, r(valsT[:, k, lb * PART:(lb + 1) * PART]),
                                r(wv_sb[:, k, :]),
                                start=(k == 0), stop=(k == DT - 1))
                        nc.any.tensor_tensor(
                            out=V1[:, lb, :, 0:64],
                            in0=pp.rearrange("p (h v) -> p h v", h=H),
                            in1=bv_b.rearrange("p (h v) -> p h v", h=H),
                            op=OP.add)
                    nc.vector.tensor_copy(
                        V1[:, :, :, 64:65],
                        ones_f.unsqueeze(1).unsqueeze(1).to_broadcast(
                            [PART, NB, H, 1]))

                    CtxT = cpool.tile([PART, DT, L], f32r, name=f"Ctx_{lname}")
                    mask_i = 1 if strict else 0

                    for h in range(H):
                        t, rb = h // 2, 64 * (h % 2)
                        qrow = slice(rb, rb + 64)
                        E = epool.tile([PART, NB, L], f32r, tag="E")
                        for j in range(NB):
                            for c in range(LC):
                                if j > 4 * c + 3:
                                    continue
                                off = max(0, j * PART - c * 512)
                                q0 = c * 512 + off
                                sp = psum_s.tile([PART, 512], f32, tag="s")
                                nc.tensor.matmul(
                                    sp[:, off:],
                                    r(GT[qrow, t, j * PART:(j + 1) * PART]),
                                    r(GT[qrow, t, q0:(c + 1) * 512]),
                                    start=True, stop=True)
                                nc.scalar.activation(
                                    out=E[:, j, q0:(c + 1) * 512], in_=sp[:, off:],
                                    func=AF.Exp, scale=0.125)
                        # mask the 8 diagonal blocks in two strided ops;
                        # diag of tile j sits at flat col 1152*j (j*1024+j*128)
                        mbc = masks[:, mask_i, :].unsqueeze(1).to_broadcast(
                            [PART, 4, PART])
                        for g in range(2):
                            dbase = E[:, 4 * g, 4 * g * PART:]
                            diag = bass.AP(
                                tensor=dbase.tensor, offset=dbase.offset,
                                ap=[list(dbase.ap[0]), [1152, 4], [1, PART]])
                            nc.gpsimd.tensor_mul(diag, diag, mbc)

                        pvs = [psum_pv.tile([PART, 512], f32, tag="pv",
                                             name=f"pv{c}")
                               for c in range(LC)]
                        for j in range(NB):
                            for c in range(LC):
                                if j > 4 * c + 3:
                                    continue
                                off = max(0, j * PART - c * 512)
                                q0 = c * 512 + off
                                nc.tensor.matmul(
                                    pvs[c][0:65, off:],
                                    r(V1[:, j, h, :]),
                                    r(E[:, j, q0:(c + 1) * 512]),
                                    start=(j == 0), stop=(j == 4 * c + 3))
                        for c in range(LC):
                            pvp = pvs[c]
                            S1 = rpool.tile([PART, 512], f32, tag="S1")
                            nc.scalar.copy(out=S1[64:65, :], in_=pvp[64:65, :])
                            Rb = rpool.tile([PART, 512], f32, tag="Rb")
                            dram_bcast(Rb[0:64, :], S1[64:65, :], 64)
                            nc.vector.reciprocal_approx_fast(
                                out=Rb[0:64, :], in_=Rb[0:64, :])
                            if strict and c == 0:
                                nc.vector.memset(Rb[0:64, 0:1], 0.0)
                            cs512 = slice(c * 512, (c + 1) * 512)
                            if h % 2 == 0:
                                nc.vector.tensor_mul(
                                    CtxT[0:64, t, cs512],
                                    pvp[0:64, :], Rb[0:64, :])
                            else:
                                ctmp = rpool.tile([PART, 512], f32r, tag="ctmp")
                                nc.vector.tensor_mul(
                                    ctmp[0:64, :], pvp[0:64, :], Rb[0:64, :])
                                nc.sync.dma_start(
                                    out=CtxT[64:128, t, cs512],
                                    in_=ctmp[0:64, :])

                    # O projection + bias + residual -> s in GT (scratch)
                    for m in range(DT):
                        pps = [psum_proj.tile([PART, 512], f32, tag="proj",
                                               name=f"pp{c}")
                               for c in range(LC)]
                        for k in range(DT):
                            for c in range(LC):
                                nc.tensor.matmul(
                                    pps[c], r(wo_sb[:, k, m * PART:(m + 1) * PART]),
                                    r(CtxT[:, k, c * 512:(c + 1) * 512]),
                                    start=(k == 0), stop=(k == DT - 1))
                        for c in range(LC):
                            cs = slice(c * 512, (c + 1) * 512)
                            nc.vector.scalar_tensor_tensor(
                                out=GT[:, m, cs], in0=pps[c],
                                scalar=bo_sb[:, m:m + 1], in1=strm[:, m, cs],
                                op0=OP.add, op1=OP.add)

                # ---- LN1: strm = LN(GT) ----
                with ExitStack() as nlc:
                    layer_norm(nlc, li, 0, GT, strm, f"{lname}a")

                # ---- FFN ----
                if ffn:
                    with ExitStack() as fc:
                        w1p = fc.enter_context(tc.tile_pool(name=f"w1_{lname}", bufs=3))
                        w2p = fc.enter_context(tc.tile_pool(name=f"w2_{lname}", bufs=1))
                        fp = fc.enter_context(tc.tile_pool(name=f"f_{lname}", bufs=1))
                        b1p = fc.enter_context(tc.tile_pool(name=f"b1_{lname}", bufs=1))
                        psum_o2 = fc.enter_context(
                            tc.tile_pool(name=f"po2_{lname}", bufs=2, space="PSUM"))

                        w2_sb = w2p.tile([PART, FT, D], f32r, tag="w2")
                        nc.sync.dma_start(
                            out=w2_sb, in_=w2_d[fi].rearrange("(k p) n -> p k n", p=PART))
                        b1_sb = b1p.tile([PART, FT], f32, tag="b1s")
                        b2_sb = b1p.tile([PART, DT], f32, tag="b2s")
                        nc.sync.dma_start(out=b1_sb, in_=b1_d[fi].rearrange("(k p) -> p k", p=PART))
                        nc.sync.dma_start(out=b2_sb, in_=b2_d[fi].rearrange("(k p) -> p k", p=PART))

                        F = fp.tile([PART, FT, L], f32r, name=f"F_{lname}")
                        for k in range(FT):
                            w1t = w1p.tile([PART, DT, PART], f32r, tag="w1")
                            nc.sync.dma_start(
                                out=w1t,
                                in_=w1_d[fi, :, k * PART:(k + 1) * PART]
                                .rearrange("(d p) n -> p d n", p=PART))
                            pps = [psum_proj.tile([PART, 512], f32,
                                                  tag="proj", name=f"fp{c}")
                                   for c in range(LC)]
                            for d in range(DT):
                                for c in range(LC):
                                    nc.tensor.matmul(
                                        pps[c], r(w1t[:, d, :]),
                                        r(strm[:, d, c * 512:(c + 1) * 512]),
                                        start=(d == 0), stop=(d == DT - 1))
                            for c in range(LC):
                                nc.vector.tensor_scalar(
                                    out=F[:, k, c * 512:(c + 1) * 512],
                                    in0=pps[c],
                                    scalar1=b1_sb[:, k:k + 1], scalar2=0.0,
                                    op0=OP.add, op1=OP.max)
                        for m in range(DT):
                            op2s = [psum_o2.tile([PART, 512], f32, tag="o2",
                                                 name=f"o2{c}")
                                    for c in range(LC)]
                            for k in range(FT):
                                for c in range(LC):
                                    nc.tensor.matmul(
                                        op2s[c],
                                        r(w2_sb[:, k, m * PART:(m + 1) * PART]),
                                        r(F[:, k, c * 512:(c + 1) * 512]),
                                        start=(k == 0), stop=(k == FT - 1))
                            for c in range(LC):
                                cs = slice(c * 512, (c + 1) * 512)
                                nc.vector.scalar_tensor_tensor(
                                    out=GT[:, m, cs], in0=op2s[c],
                                    scalar=b2_sb[:, m:m + 1], in1=strm[:, m, cs],
                                    op0=OP.add, op1=OP.add)
                    with ExitStack() as nlc:
                        layer_norm(nlc, li, 1, GT, strm, f"{lname}b")

        nc.sync.dma_start(out=out_d.rearrange("(t p) l -> p t l", p=PART), in_=xT)


_LDW_PATCHED = [False]


def _maybe_patch_ldw_opt():
    """Enable walrus LDWEIGHTS dedup (--enable-ldw-opt) so repeated
    stationary operands aren't reloaded; gated by KERNEL_LDWOPT env."""
    import os
    if _LDW_PATCHED[0] or os.environ.get("KERNEL_LDWOPT", "1") != "1":
        return
    import concourse.bass_utils as bu
    orig = bu.run_command

    def patched(cmd, **kw):
        cmd = ["--enable-ldw-opt=true" if c == "--enable-ldw-opt=false" else c
               for c in cmd]
        return orig(cmd, **kw)

    bu.run_command = patched
    _LDW_PATCHED[0] = True


def _build(ln_degenerate):
    import concourse.bacc as bacc

    _maybe_patch_ldw_opt()
    key = ("nc", ln_degenerate)
    if key in _CACHE:
        return _CACHE[key]
    nc = bacc.Bacc("TRN2", target_bir_lowering=False, debug=False,
                   num_devices=NCORES)
    _emit(nc, ln_degenerate)
    nc.compile()
    _CACHE[key] = nc
    return nc


def _prepare(inputs):
    q = np.ascontiguousarray(np.asarray(inputs["q_embed_data"], dtype=np.float32))
    qa = np.ascontiguousarray(np.asarray(inputs["qa_embed_data"], dtype=np.float32))
    p = inputs["params"]

    def g(block, name):
        return np.asarray(p[block][name], dtype=np.float32)

    def stack6(name):
        return np.ascontiguousarray(
            np.concatenate([g("b1", name), g("b2", name)], axis=0))

    wk, wv, wo = stack6("Wk"), stack6("Wv"), stack6("Wo")
    bk, bv, bo = stack6("bk"), stack6("bv"), stack6("bo")
    lnw = np.ascontiguousarray(np.stack([stack6("ln1_w"), stack6("ln2_w")], axis=1))
    lnb = np.ascontiguousarray(np.stack([stack6("ln1_b"), stack6("ln2_b")], axis=1))
    ffn_layers = [gl for gl, cfg in enumerate(LAYERS) if cfg[3]]
    w1 = np.ascontiguousarray(stack6("W1")[ffn_layers])
    b1 = np.ascontiguousarray(stack6("b1")[ffn_layers])
    w2 = np.ascontiguousarray(stack6("W2")[ffn_layers])
    b2 = np.ascontiguousarray(stack6("b2")[ffn_layers])

    ln_degenerate = bool(np.all(lnw == 1.0) and np.all(lnb == 0.0))

    shared = dict(wk=wk, wv=wv, wo=wo, bk=bk, bv=bv, bo=bo,
                  lnw=lnw, lnb=lnb, w1=w1, b1=b1, w2=w2, b2=b2)
    in_maps = []
    for b in range(B):
        m = dict(shared)
        m["xT"] = np.ascontiguousarray(q[b].T)
        m["yT"] = np.ascontiguousarray(qa[b].T)
        in_maps.append(m)
    return in_maps, ln_degenerate


def run(inputs, trace=False):
    from concourse.bass_utils import run_bass_kernel_spmd

    in_maps, ln_degenerate = _prepare(inputs)
    nc = _build(ln_degenerate)
    res = run_bass_kernel_spmd(nc, in_maps, core_ids=list(range(NCORES)),
                               trace=trace)
    out = np.stack([res.results[b]["outT"].T for b in range(B)])
    return out, res


def kernel(**inputs):
    out, _ = run(inputs, trace=False)
    return out


# revision 23
# speedup vs baseline: 1.0202x; 1.0202x over previous
"""Trainium2 Bass kernel for the CFGKT dense transformer (B=8, L=1024, D=512,
H=8, DFF=2048; 2 self-attn+FFN layers on qa_embed, then 4 layers on q_embed
alternating self-attn and cross-attn-to-y).

Sharding: pure data-parallel — one batch element per NeuronCore, zero
collectives.  Inside each core everything runs on a transposed activation
layout ([D, L], d on partitions) so projections are plain lhsT=W matmuls.

Key algorithmic points (validated vs reference in numpy):
  - kq_same=True and query==key input in every layer, so scores S = Q @ Q^T are
    symmetric.  We compute only upper-triangle-by-block tiles of E = exp(S/8)
    ([k-part, q-free] layout) and use each tile both for the row-softmax
    denominator and as the PV right operand — no transposes anywhere.
  - Softmax without max-subtraction (|S/8| is small), denominators via a ones
    column folded into the V stationary operand (even heads: [V|1], odd heads:
    [1|pad|V] with tile_position=(0,32) so ctx rows land partition-aligned).
  - Strictly-causal layers (mask_k=0): row q=0 fully masked -> reciprocal row
    gets column 0 forced to 0 after the reciprocal, giving exactly 0 output.
  - LayerNorm stats over the partition dim via ones-vector matmuls (sum and
    sum-of-squares), applied as (D*x - musum) * rD with
    rD = 1/sqrt(D^2*(var+eps)); per-column vectors are replicated across
    partitions by DMA broadcast.
  - All matmuls run as float32r (full fp32 data, 1 cycle/row at N>=256).
"""

import numpy as np

B, L, D, H, DFF = 8, 1024, 512, 8, 2048
DK = D // H          # 64
PART = 128
DT = D // PART       # 4 d-tiles
NB = L // PART       # 8 L-blocks of 128
LC = L // 512        # 2 L-chunks of 512
FT = DFF // PART     # 16 ff tiles
NCORES = 8
EPS = 1e-5

# layer configs: (stream, vals, strict, ffn, ffn_idx)
LAYERS = [
    ("y", "self", False, True, 0),
    ("y", "self", False, True, 1),
    ("x", "self", False, False, None),
    ("x", "y", True, True, 2),
    ("x", "self", False, False, None),
    ("x", "y", True, True, 3),
]
NFFN = 4

_CACHE = {}


def _emit(nc, ln_degenerate):
    import concourse.bass as bass
    import concourse.tile as tile
    from concourse import mybir
    from contextlib import ExitStack

    f32 = mybir.dt.float32
    f32r = mybir.dt.float32r
    AF = mybir.ActivationFunctionType
    OP = mybir.AluOpType

    def r(ap):
        return ap

    # ---- DRAM I/O ----
    xT_d = nc.dram_tensor("xT", [D, L], f32r, kind="ExternalInput").ap()
    yT_d = nc.dram_tensor("yT", [D, L], f32r, kind="ExternalInput").ap()
    wk_d = nc.dram_tensor("wk", [6, D, D], f32r, kind="ExternalInput").ap()
    wv_d = nc.dram_tensor("wv", [6, D, D], f32r, kind="ExternalInput").ap()
    wo_d = nc.dram_tensor("wo", [6, D, D], f32r, kind="ExternalInput").ap()
    bk_d = nc.dram_tensor("bk", [6, D], f32, kind="ExternalInput").ap()
    bv_d = nc.dram_tensor("bv", [6, D], f32, kind="ExternalInput").ap()
    bo_d = nc.dram_tensor("bo", [6, D], f32, kind="ExternalInput").ap()
    lnw_d = nc.dram_tensor("lnw", [6, 2, D], f32, kind="ExternalInput").ap()
    lnb_d = nc.dram_tensor("lnb", [6, 2, D], f32, kind="ExternalInput").ap()
    w1_d = nc.dram_tensor("w1", [NFFN, D, DFF], f32r, kind="ExternalInput").ap()
    b1_d = nc.dram_tensor("b1", [NFFN, DFF], f32, kind="ExternalInput").ap()
    w2_d = nc.dram_tensor("w2", [NFFN, DFF, D], f32r, kind="ExternalInput").ap()
    b2_d = nc.dram_tensor("b2", [NFFN, D], f32, kind="ExternalInput").ap()
    out_d = nc.dram_tensor("outT", [D, L], f32r, kind="ExternalOutput").ap()

    # masks embedded in the NEFF: [2, 128, 128]; idx 0 causal (k<=q),
    # idx 1 strict (k<q) — multiplicative 0/1, applied post-exp.
    kk = np.arange(PART)[:, None]
    qq = np.arange(PART)[None, :]
    masks_np = np.stack([(kk <= qq), (kk < qq)]).astype(np.float32)
    masks_d = nc.inline_tensor(masks_np, name="masks").ap()

    with tile.TileContext(nc) as tc, ExitStack() as ctx:
        pers = ctx.enter_context(tc.tile_pool(name="pers", bufs=1))
        wkpool = ctx.enter_context(tc.tile_pool(name="wkpool", bufs=2))
        wpool = ctx.enter_context(tc.tile_pool(name="wpool", bufs=1))

        xT = pers.tile([PART, DT, L], f32r, name="xT_sb")
        yT = pers.tile([PART, DT, L], f32r, name="yT_sb")
        masks = pers.tile([PART, 2, PART], f32, name="masks_sb")
        ones = pers.tile([PART, 1], f32r, name="ones_sb")
        epsc = pers.tile([1, 1], f32, name="eps_sb")
        nc.vector.memset(epsc, float(D) * float(D) * EPS)
        # memset can't write f32r; stage 1.0 in f32 and copy (engine rounds)
        ones_f = pers.tile([PART, 1], f32, name="ones_f_sb")
        nc.vector.memset(ones_f, 1.0)
        ones_row = pers.tile([1, PART], f32r, name="ones_row_sb")
        nc.vector.tensor_copy(ones_row, ones_f[0:1, 0:1].to_broadcast([1, PART]))

        # DRAM scratch used to replicate per-column vectors across partitions
        # (SBUF->DRAM then DRAM->SBUF with a step-0 partition source).
        scr_d = nc.dram_tensor("bscr", [256, 512], f32).ap()
        slot_ctr = [0]

        def dram_bcast(dst_ap, src_ap, nparts):
            s = slot_ctr[0]
            slot_ctr[0] += 1
            nc.sync.dma_start(out=scr_d[s:s + 1, :], in_=src_ap)
            nc.sync.dma_start(
                out=dst_ap, in_=scr_d[s:s + 1, :].to_broadcast([nparts, 512]))

        nc.sync.dma_start(out=xT, in_=xT_d.rearrange("(t p) l -> p t l", p=PART))
        nc.sync.dma_start(out=yT, in_=yT_d.rearrange("(t p) l -> p t l", p=PART))
        nc.sync.dma_start(out=masks, in_=masks_d.rearrange("i p q -> p i q"))
        nc.vector.tensor_copy(ones, ones_f)

        cur = {}

        def layer_norm(lc, li, which, src, dst, lname):
            """LN over the partition dim of the transposed stream.
            src/dst: sbuf [128, DT, L]."""
            psum_st = lc.enter_context(
                tc.tile_pool(name=f"pst_{lname}", bufs=2, space="PSUM"))
            spool = lc.enter_context(tc.tile_pool(name=f"sq_{lname}", bufs=2))
            bpool = lc.enter_context(tc.tile_pool(name=f"lnb_{lname}", bufs=2))
            for c in range(LC):
                cs = slice(c * 512, (c + 1) * 512)
                sq = spool.tile([PART, DT, 512], f32r, tag="sq")
                for k in range(DT):
                    nc.scalar.square(out=sq[:, k, :], in_=src[:, k, cs])
                mu = psum_st.tile([1, 512], f32, tag="mu", bufs=1)
                ssq = psum_st.tile([1, 512], f32, tag="ssq", bufs=1)
                for k in range(DT):
                    nc.tensor.matmul(mu, r(ones), r(src[:, k, cs]),
                                     start=(k == 0), stop=(k == DT - 1))
                for k in range(DT):
                    nc.tensor.matmul(ssq, r(ones), r(sq[:, k, :]),
                                     start=(k == 0), stop=(k == DT - 1))
                m2 = spool.tile([1, 512], f32, tag="m2")
                nc.scalar.square(out=m2, in_=mu[0:1, :])
                vD2 = spool.tile([1, 512], f32, tag="vD2")
                nc.vector.scalar_tensor_tensor(
                    out=vD2, in0=ssq[0:1, :], scalar=float(D), in1=m2,
                    op0=OP.mult, op1=OP.subtract)
                # t = sqrt(vD2 + D^2*eps);  rD = 1/t = rstd/D
                tsq = spool.tile([1, 512], f32r, tag="tsq")
                nc.scalar.activation(out=tsq, in_=vD2, func=AF.Sqrt,
                                     bias=epsc, scale=1.0)
                # replicate mu and tsq across partitions via PE (ones-row
                # outer products into psum); rD_b = 1/tsq via fast approx
                mu_sb = spool.tile([1, 512], f32r, tag="mu_sb")
                nc.scalar.copy(out=mu_sb, in_=mu[0:1, :])
                mu_b = psum_st.tile([PART, 512], f32, tag="mu_b", bufs=2)
                nc.tensor.matmul(mu_b, r(ones_row), r(mu_sb),
                                 start=True, stop=True)
                ts_b = psum_st.tile([PART, 512], f32, tag="ts_b", bufs=2)
                nc.tensor.matmul(ts_b, r(ones_row), r(tsq),
                                 start=True, stop=True)
                rD_b = bpool.tile([PART, 512], f32, tag="rD_b")
                nc.vector.reciprocal_approx_fast(out=rD_b, in_=ts_b)
                for k in range(DT):
                    t1 = spool.tile([PART, 512], f32, tag="t1")
                    nc.vector.scalar_tensor_tensor(
                        out=t1, in0=src[:, k, cs], scalar=float(D), in1=mu_b,
                        op0=OP.mult, op1=OP.subtract)
                    if ln_degenerate:
                        nc.vector.tensor_mul(dst[:, k, cs], t1, rD_b)
                    else:
                        t2 = spool.tile([PART, 512], f32, tag="t2")
                        nc.vector.tensor_mul(t2, t1, rD_b)
                        nc.vector.tensor_scalar(
                            out=dst[:, k, cs], in0=t2,
                            scalar1=cur["lnw"][:, which, li:li + 1],
                            scalar2=cur["lnb"][:, which, li:li + 1],
                            op0=OP.mult, op1=OP.add)

        for li, (strm_name, vals_name, strict, ffn, fi) in enumerate(LAYERS):
            strm = yT if strm_name == "y" else xT
            valsT = yT if vals_name == "y" else strm
            lname = f"l{li}"
            with ExitStack() as lc:
                psum_proj = lc.enter_context(
                    tc.tile_pool(name=f"ppj_{lname}", bufs=2, space="PSUM"))
                gt_pool = lc.enter_context(tc.tile_pool(name=f"gt_{lname}", bufs=1))
                GT = gt_pool.tile([PART, DT, L], f32r, name=f"GT_{lname}")

                wk_sb = wkpool.tile([PART, DT, D], f32r, tag="wk", name=f"wk_{lname}")
                wv_sb = wpool.tile([PART, DT, D], f32r, tag="wv", name=f"wv_{lname}")
                wo_sb = wpool.tile([PART, DT, D], f32r, tag="wo", name=f"wo_{lname}")
                bk_sb = wpool.tile([PART, DT], f32, tag="bk", name=f"bk_{lname}")
                bo_sb = wpool.tile([PART, DT], f32, tag="bo", name=f"bo_{lname}")
                bv_b = wpool.tile([PART, D], f32, tag="bv_b", name=f"bvb_{lname}")
                nc.sync.dma_start(out=wk_sb, in_=wk_d[li].rearrange("(k p) n -> p k n", p=PART))
                nc.sync.dma_start(out=wv_sb, in_=wv_d[li].rearrange("(k p) n -> p k n", p=PART))
                nc.sync.dma_start(out=wo_sb, in_=wo_d[li].rearrange("(k p) n -> p k n", p=PART))
                nc.sync.dma_start(out=bk_sb, in_=bk_d[li].rearrange("(k p) -> p k", p=PART))
                nc.sync.dma_start(out=bo_sb, in_=bo_d[li].rearrange("(k p) -> p k", p=PART))
                nc.sync.dma_start(
                    out=bv_b,
                    in_=bv_d[li].unsqueeze(0).to_broadcast([PART, D]))
                if not ln_degenerate:
                    lnw_sb = wpool.tile([PART, 2, DT], f32, tag="lnw", name=f"lnw_{lname}")
                    lnb_sb = wpool.tile([PART, 2, DT], f32, tag="lnb", name=f"lnb_{lname}")
                    nc.sync.dma_start(out=lnw_sb, in_=lnw_d[li].rearrange("i (k p) -> p i k", p=PART))
                    nc.sync.dma_start(out=lnb_sb, in_=lnb_d[li].rearrange("i (k p) -> p i k", p=PART))
                    cur["lnw"], cur["lnb"] = lnw_sb, lnb_sb

                # ---- attention ----
                with ExitStack() as ac:
                    psum_s = ac.enter_context(
                        tc.tile_pool(name=f"ps_{lname}", bufs=3, space="PSUM"))
                    psum_pv = ac.enter_context(
                        tc.tile_pool(name=f"ppv_{lname}", bufs=3, space="PSUM"))
                    epool = ac.enter_context(tc.tile_pool(name=f"ep_{lname}", bufs=2))
                    vpool = ac.enter_context(tc.tile_pool(name=f"vp_{lname}", bufs=1))
                    cpool = ac.enter_context(tc.tile_pool(name=f"cp_{lname}", bufs=1))
                    rpool = ac.enter_context(tc.tile_pool(name=f"rp_{lname}", bufs=3))

                    # QK projection: GT = Wk^T @ strm + bk  (transposed out)
                    # weight-stationary: both L-chunks consume one LDWEIGHTS
                    for c in range(LC):
                        for m in range(DT):
                            pp = psum_proj.tile([PART, 512], f32, tag="proj",
                                                name=f"pp{c}_{m}")
                            for k in range(DT):
                                nc.tensor.matmul(
                                    pp, r(wk_sb[:, k, m * PART:(m + 1) * PART]),
                                    r(strm[:, k, c * 512:(c + 1) * 512]),
                                    start=(k == 0), stop=(k == DT - 1))
                            nc.vector.tensor_scalar(
                                out=GT[:, m, c * 512:(c + 1) * 512], in0=pp,
                                scalar1=bk_sb[:, m:m + 1], scalar2=None,
                                op0=OP.add)

                    # V projection (natural layout + ones col per head):
                    # V1[:, lb, h, 0:64] = V rows of block lb, head h; col 64 = 1
                    V1 = vpool.tile([PART, NB, H, 65], f32r, name=f"V1_{lname}")
                    for lb in range(NB):
                        pp = psum_proj.tile([PART, 512], f32, tag="proj")
                        for k in range(DT):
                            nc.tensor.matmul(
                                pp, r(valsT[:, k, lb * PART:(lb + 1) * PART]),
                                r(wv_sb[:, k, :]),
                                start=(k == 0), stop=(k == DT - 1))
                        nc.any.tensor_tensor(
                            out=V1[:, lb, :, 0:64],
                            in0=pp.rearrange("p (h v) -> p h v", h=H),
                            in1=bv_b.rearrange("p (h v) -> p h v", h=H),
                            op=OP.add)
                    nc.vector.tensor_copy(
                        V1[:, :, :, 64:65],
                        ones_f.unsqueeze(1).unsqueeze(1).to_broadcast(
                            [PART, NB, H, 1]))

                    CtxT = cpool.tile([PART, DT, L], f32r, name=f"Ctx_{lname}")
                    mask_i = 1 if strict else 0

                    for h in range(H):
                        t, rb = h // 2, 64 * (h % 2)
                        qrow = slice(rb, rb + 64)
                        E = epool.tile([PART, NB, L], f32r, tag="E")
                        for j in range(NB):
                            for c in range(LC):
                                if j > 4 * c + 3:
                                    continue
                                off = max(0, j * PART - c * 512)
                                q0 = c * 512 + off
                                sp = psum_s.tile([PART, 512], f32, tag="s")
                                nc.tensor.matmul(
                                    sp[:, off:],
                                    r(GT[qrow, t, j * PART:(j + 1) * PART]),
                                    r(GT[qrow, t, q0:(c + 1) * 512]),
                                    start=True, stop=True)
                                nc.scalar.activation(
                                    out=E[:, j, q0:(c + 1) * 512], in_=sp[:, off:],
                                    func=AF.Exp, scale=0.125)
                        # mask the 8 diagonal blocks in two strided ops;
                        # diag of tile j sits at flat col 1152*j (j*1024+j*128)
                        mbc = masks[:, mask_i, :].unsqueeze(1).to_broadcast(
                            [PART, 4, PART])
                        for g in range(2):
                            dbase = E[:, 4 * g, 4 * g * PART:]
                            diag = bass.AP(
                                tensor=dbase.tensor, offset=dbase.offset,
                                ap=[list(dbase.ap[0]), [1152, 4], [1, PART]])
                            nc.gpsimd.tensor_mul(diag, diag, mbc)

                        pvs = [psum_pv.tile([PART, 512], f32, tag="pv",
                                             name=f"pv{c}")
                               for c in range(LC)]
                        for j in range(NB):
                            for c in range(LC):
                                if j > 4 * c + 3:
                                    continue
                                off = max(0, j * PART - c * 512)
                                q0 = c * 512 + off
                                nc.tensor.matmul(
                                    pvs[c][0:65, off:],
                                    r(V1[:, j, h, :]),
                                    r(E[:, j, q0:(c + 1) * 512]),
                                    start=(j == 0), stop=(j == 4 * c + 3))
                        for c in range(LC):
                            pvp = pvs[c]
                            S1 = rpool.tile([PART, 512], f32, tag="S1")
                            nc.scalar.copy(out=S1[64:65, :], in_=pvp[64:65, :])
                            Rb = rpool.tile([PART, 512], f32, tag="Rb")
                            dram_bcast(Rb[0:64, :], S1[64:65, :], 64)
                            nc.vector.reciprocal_approx_fast(
                                out=Rb[0:64, :], in_=Rb[0:64, :])
                            if strict and c == 0:
                                nc.vector.memset(Rb[0:64, 0:1], 0.0)
                            cs512 = slice(c * 512, (c + 1) * 512)
                            if h % 2 == 0:
                                nc.vector.tensor_mul(
                                    CtxT[0:64, t, cs512],
                                    pvp[0:64, :], Rb[0:64, :])
                            else:
                                ctmp = rpool.tile([PART, 512], f32r, tag="ctmp")
                                nc.vector.tensor_mul(
                                    ctmp[0:64, :], pvp[0:64, :], Rb[0:64, :])
                                nc.sync.dma_start(
                                    out=CtxT[64:128, t, cs512],
                                    in_=ctmp[0:64, :])

                    # O projection + bias + residual -> s in GT (scratch)
                    for m in range(DT):
                        pps = [psum_proj.tile([PART, 512], f32, tag="proj",
                                               name=f"pp{c}")
                               for c in range(LC)]
                        for k in range(DT):
                            for c in range(LC):
                                nc.tensor.matmul(
                                    pps[c], r(wo_sb[:, k, m * PART:(m + 1) * PART]),
                                    r(CtxT[:, k, c * 512:(c + 1) * 512]),
                                    start=(k == 0), stop=(k == DT - 1))
                        for c in range(LC):
                            cs = slice(c * 512, (c + 1) * 512)
                            nc.vector.scalar_tensor_tensor(
                                out=GT[:, m, cs], in0=pps[c],
                                scalar=bo_sb[:, m:m + 1], in1=strm[:, m, cs],
                                op0=OP.add, op1=OP.add)

                # ---- LN1: strm = LN(GT) ----
                with ExitStack() as nlc:
                    layer_norm(nlc, li, 0, GT, strm, f"{lname}a")

                # ---- FFN ----
                if ffn:
                    with ExitStack() as fc:
                        w1p = fc.enter_context(tc.tile_pool(name=f"w1_{lname}", bufs=3))
                        w2p = fc.enter_context(tc.tile_pool(name=f"w2_{lname}", bufs=1))
                        fp = fc.enter_context(tc.tile_pool(name=f"f_{lname}", bufs=1))
                        b1p = fc.enter_context(tc.tile_pool(name=f"b1_{lname}", bufs=1))
                        psum_o2 = fc.enter_context(
                            tc.tile_pool(name=f"po2_{lname}", bufs=2, space="PSUM"))

                        w2_sb = w2p.tile([PART, FT, D], f32r, tag="w2")
                        nc.sync.dma_start(
                            out=w2_sb, in_=w2_d[fi].rearrange("(k p) n -> p k n", p=PART))
                        b1_sb = b1p.tile([PART, FT], f32, tag="b1s")
                        b2_sb = b1p.tile([PART, DT], f32, tag="b2s")
                        nc.sync.dma_start(out=b1_sb, in_=b1_d[fi].rearrange("(k p) -> p k", p=PART))
                        nc.sync.dma_start(out=b2_sb, in_=b2_d[fi].rearrange("(k p) -> p k", p=PART))

                        F = fp.tile([PART, FT, L], f32r, name=f"F_{lname}")
                        for k in range(FT):
                            w1t = w1p.tile([PART, DT, PART], f32r, tag="w1")
                            nc.sync.dma_start(
                                out=w1t,
                                in_=w1_d[fi, :, k * PART:(k + 1) * PART]
                                .rearrange("(d p) n -> p d n", p=PART))
                            pps = [psum_proj.tile([PART, 512], f32,
                                                  tag="proj", name=f"fp{c}")
                                   for c in range(LC)]
                            for d in range(DT):
                                for c in range(LC):
                                    nc.tensor.matmul(
                                        pps[c], r(w1t[:, d, :]),
                                        r(strm[:, d, c * 512:(c + 1) * 512]),
                                        start=(d == 0), stop=(d == DT - 1))
                            for c in range(LC):
                                nc.vector.tensor_scalar(
                                    out=F[:, k, c * 512:(c + 1) * 512],
                                    in0=pps[c],
                                    scalar1=b1_sb[:, k:k + 1], scalar2=0.0,
                                    op0=OP.add, op1=OP.max)
                        for m in range(DT):
                            op2s = [psum_o2.tile([PART, 512], f32, tag="o2",
                                                 name=f"o2{c}")
                                    for c in range(LC)]
                            for k in range(FT):
                                for c in range(LC):
                                    nc.tensor.matmul(
                                        op2s[c],
                                        r(w2_sb[:, k, m * PART:(m + 1) * PART]),
                                        r(F[:, k, c * 512:(c + 1) * 512]),
                                        start=(k == 0), stop=(k == FT - 1))
                            for c in range(LC):
                                cs = slice(c * 512, (c + 1) * 512)
                                nc.vector.scalar_tensor_tensor(
                                    out=GT[:, m, cs], in0=op2s[c],
                                    scalar=b2_sb[:, m:m + 1], in1=strm[:, m, cs],
                                    op0=OP.add, op1=OP.add)
                    with ExitStack() as nlc:
                        layer_norm(nlc, li, 1, GT, strm, f"{lname}b")

        nc.sync.dma_start(out=out_d.rearrange("(t p) l -> p t l", p=PART), in_=xT)


_LDW_PATCHED = [False]


def _maybe_patch_ldw_opt():
    """Enable walrus LDWEIGHTS dedup (--enable-ldw-opt) so repeated
    stationary operands aren't reloaded; gated by KERNEL_LDWOPT env."""
    import os
    if _LDW_PATCHED[0] or os.environ.get("KERNEL_LDWOPT", "1") != "1":
        return
    import concourse.bass_utils as bu
    orig = bu.run_command

    def patched(cmd, **kw):
        cmd = ["--enable-ldw-opt=true" if c == "--enable-ldw-opt=false" else c
               for c in cmd]
        return orig(cmd, **kw)

    bu.run_command = patched
    _LDW_PATCHED[0] = True


def _build(ln_degenerate):
    import concourse.bacc as bacc

    _maybe_patch_ldw_opt()
    key = ("nc", ln_degenerate)
    if key in _CACHE:
        return _CACHE[key]
    nc = bacc.Bacc("TRN2", target_bir_lowering=False, debug=False,
                   num_devices=NCORES)
    _emit(nc, ln_degenerate)
    nc.compile()
    _CACHE[key] = nc
    return nc


def _prepare(inputs):
    q = np.ascontiguousarray(np.asarray(inputs["q_embed_data"], dtype=np.float32))
    qa = np.ascontiguousarray(np.asarray(inputs["qa_embed_data"], dtype=np.float32))
    p = inputs["params"]

    def g(block, name):
        return np.asarray(p[block][name], dtype=np.float32)

    def stack6(name):
        return np.ascontiguousarray(
            np.concatenate([g("b1", name), g("b2", name)], axis=0))

    wk, wv, wo = stack6("Wk"), stack6("Wv"), stack6("Wo")
    bk, bv, bo = stack6("bk"), stack6("bv"), stack6("bo")
    lnw = np.ascontiguousarray(np.stack([stack6("ln1_w"), stack6("ln2_w")], axis=1))
    lnb = np.ascontiguousarray(np.stack([stack6("ln1_b"), stack6("ln2_b")], axis=1))
    ffn_layers = [gl for gl, cfg in enumerate(LAYERS) if cfg[3]]
    w1 = np.ascontiguousarray(stack6("W1")[ffn_layers])
    b1 = np.ascontiguousarray(stack6("b1")[ffn_layers])
    w2 = np.ascontiguousarray(stack6("W2")[ffn_layers])
    b2 = np.ascontiguousarray(stack6("b2")[ffn_layers])

    ln_degenerate = bool(np.all(lnw == 1.0) and np.all(lnb == 0.0))

    shared = dict(wk=wk, wv=wv, wo=wo, bk=bk, bv=bv, bo=bo,
                  lnw=lnw, lnb=lnb, w1=w1, b1=b1, w2=w2, b2=b2)
    in_maps = []
    for b in range(B):
        m = dict(shared)
        m["xT"] = np.ascontiguousarray(q[b].T)
        m["yT"] = np.ascontiguousarray(qa[b].T)
        in_maps.append(m)
    return in_maps, ln_degenerate


def run(inputs, trace=False):
    from concourse.bass_utils import run_bass_kernel_spmd

    in_maps, ln_degenerate = _prepare(inputs)
    nc = _build(ln_degenerate)
    res = run_bass_kernel_spmd(nc, in_maps, core_ids=list(range(NCORES)),
                               trace=trace)
    out = np.stack([res.results[b]["outT"].T for b in range(B)])
    return out, res


def kernel(**inputs):
    out, _ = run(inputs, trace=False)
    return out


# revision 26
# speedup vs baseline: 1.0309x; 1.0105x over previous
"""Trainium2 Bass kernel for the CFGKT dense transformer (B=8, L=1024, D=512,
H=8, DFF=2048; 2 self-attn+FFN layers on qa_embed, then 4 layers on q_embed
alternating self-attn and cross-attn-to-y).

Sharding: pure data-parallel — one batch element per NeuronCore, zero
collectives.  Inside each core everything runs on a transposed activation
layout ([D, L], d on partitions) so projections are plain lhsT=W matmuls.

Key algorithmic points (validated vs reference in numpy):
  - kq_same=True and query==key input in every layer, so scores S = Q @ Q^T are
    symmetric.  We compute only upper-triangle-by-block tiles of E = exp(S/8)
    ([k-part, q-free] layout) and use each tile both for the row-softmax
    denominator and as the PV right operand — no transposes anywhere.
  - Softmax without max-subtraction (|S/8| is small), denominators via a ones
    column folded into the V stationary operand (even heads: [V|1], odd heads:
    [1|pad|V] with tile_position=(0,32) so ctx rows land partition-aligned).
  - Strictly-causal layers (mask_k=0): row q=0 fully masked -> reciprocal row
    gets column 0 forced to 0 after the reciprocal, giving exactly 0 output.
  - LayerNorm stats over the partition dim via ones-vector matmuls (sum and
    sum-of-squares), applied as (D*x - musum) * rD with
    rD = 1/sqrt(D^2*(var+eps)); per-column vectors are replicated across
    partitions by DMA broadcast.
  - All matmuls run as float32r (full fp32 data, 1 cycle/row at N>=256).
"""

import numpy as np

B, L, D, H, DFF = 8, 1024, 512, 8, 2048
DK = D // H          # 64
PART = 128
DT = D // PART       # 4 d-tiles
NB = L // PART       # 8 L-blocks of 128
LC = L // 512        # 2 L-chunks of 512
FT = DFF // PART     # 16 ff tiles
NCORES = 8
EPS = 1e-5

# layer configs: (stream, vals, strict, ffn, ffn_idx)
LAYERS = [
    ("y", "self", False, True, 0),
    ("y", "self", False, True, 1),
    ("x", "self", False, False, None),
    ("x", "y", True, True, 2),
    ("x", "self", False, False, None),
    ("x", "y", True, True, 3),
]
NFFN = 4

_CACHE = {}


def _emit(nc, ln_degenerate, bv_zero=True):
    import concourse.bass as bass
    import concourse.tile as tile
    from concourse import mybir
    from contextlib import ExitStack

    f32 = mybir.dt.float32
    f32r = mybir.dt.float32r
    AF = mybir.ActivationFunctionType
    OP = mybir.AluOpType

    def r(ap):
        return ap

    # ---- DRAM I/O ----
    xT_d = nc.dram_tensor("xT", [D, L], f32r, kind="ExternalInput").ap()
    yT_d = nc.dram_tensor("yT", [D, L], f32r, kind="ExternalInput").ap()
    wk_d = nc.dram_tensor("wk", [6, D, D], f32r, kind="ExternalInput").ap()
    wv_d = nc.dram_tensor("wv", [6, D, D], f32r, kind="ExternalInput").ap()
    wo_d = nc.dram_tensor("wo", [6, D, D], f32r, kind="ExternalInput").ap()
    bk_d = nc.dram_tensor("bk", [6, D], f32, kind="ExternalInput").ap()
    bv_d = nc.dram_tensor("bv", [6, D], f32, kind="ExternalInput").ap()
    bo_d = nc.dram_tensor("bo", [6, D], f32, kind="ExternalInput").ap()
    lnw_d = nc.dram_tensor("lnw", [6, 2, D], f32, kind="ExternalInput").ap()
    lnb_d = nc.dram_tensor("lnb", [6, 2, D], f32, kind="ExternalInput").ap()
    w1_d = nc.dram_tensor("w1", [NFFN, D, DFF], f32r, kind="ExternalInput").ap()
    b1_d = nc.dram_tensor("b1", [NFFN, DFF], f32, kind="ExternalInput").ap()
    w2_d = nc.dram_tensor("w2", [NFFN, DFF, D], f32r, kind="ExternalInput").ap()
    b2_d = nc.dram_tensor("b2", [NFFN, D], f32, kind="ExternalInput").ap()
    out_d = nc.dram_tensor("outT", [D, L], f32r, kind="ExternalOutput").ap()

    # masks embedded in the NEFF: [2, 128, 128]; idx 0 causal (k<=q),
    # idx 1 strict (k<q) — multiplicative 0/1, applied post-exp.
    kk = np.arange(PART)[:, None]
    qq = np.arange(PART)[None, :]
    masks_np = np.stack([(kk <= qq), (kk < qq)]).astype(np.float32)
    masks_d = nc.inline_tensor(masks_np, name="masks").ap()

    with tile.TileContext(nc) as tc, ExitStack() as ctx:
        pers = ctx.enter_context(tc.tile_pool(name="pers", bufs=1))
        wkpool = ctx.enter_context(tc.tile_pool(name="wkpool", bufs=2))
        wpool = ctx.enter_context(tc.tile_pool(name="wpool", bufs=1))
        gt_pool = ctx.enter_context(tc.tile_pool(name="gt_g", bufs=2))
        psum_proj = ctx.enter_context(
            tc.tile_pool(name="ppj_g", bufs=2, space="PSUM"))

        xT = pers.tile([PART, DT, L], f32r, name="xT_sb")
        yT = pers.tile([PART, DT, L], f32r, name="yT_sb")
        masks = pers.tile([PART, 2, PART], f32, name="masks_sb")
        ones = pers.tile([PART, 1], f32r, name="ones_sb")
        epsc = pers.tile([1, 1], f32, name="eps_sb")
        nc.vector.memset(epsc, float(D) * float(D) * EPS)
        # memset can't write f32r; stage 1.0 in f32 and copy (engine rounds)
        ones_f = pers.tile([PART, 1], f32, name="ones_f_sb")
        nc.vector.memset(ones_f, 1.0)
        ones_row = pers.tile([1, PART], f32r, name="ones_row_sb")
        nc.vector.tensor_copy(ones_row, ones_f[0:1, 0:1].to_broadcast([1, PART]))

        # DRAM scratch used to replicate per-column vectors across partitions
        # (SBUF->DRAM then DRAM->SBUF with a step-0 partition source).
        scr_d = nc.dram_tensor("bscr", [256, 512], f32).ap()
        slot_ctr = [0]

        def dram_bcast(dst_ap, src_ap, nparts):
            s = slot_ctr[0]
            slot_ctr[0] += 1
            nc.sync.dma_start(out=scr_d[s:s + 1, :], in_=src_ap)
            nc.sync.dma_start(
                out=dst_ap, in_=scr_d[s:s + 1, :].to_broadcast([nparts, 512]))

        nc.sync.dma_start(out=xT, in_=xT_d.rearrange("(t p) l -> p t l", p=PART))
        nc.sync.dma_start(out=yT, in_=yT_d.rearrange("(t p) l -> p t l", p=PART))
        nc.sync.dma_start(out=masks, in_=masks_d.rearrange("i p q -> p i q"))
        nc.vector.tensor_copy(ones, ones_f)

        cur = {}

        def layer_norm(lc, li, which, src, dst, lname):
            """LN over the partition dim of the transposed stream.
            src/dst: sbuf [128, DT, L]."""
            psum_st = lc.enter_context(
                tc.tile_pool(name=f"pst_{lname}", bufs=2, space="PSUM"))
            spool = lc.enter_context(tc.tile_pool(name=f"sq_{lname}", bufs=2))
            bpool = lc.enter_context(tc.tile_pool(name=f"lnb_{lname}", bufs=2))
            for c in range(LC):
                cs = slice(c * 512, (c + 1) * 512)
                sq = spool.tile([PART, DT, 512], f32r, tag="sq")
                for k in range(DT):
                    nc.scalar.square(out=sq[:, k, :], in_=src[:, k, cs])
                mu = psum_st.tile([1, 512], f32, tag="mu", bufs=1)
                ssq = psum_st.tile([1, 512], f32, tag="ssq", bufs=1)
                for k in range(DT):
                    nc.tensor.matmul(mu, r(ones), r(src[:, k, cs]),
                                     start=(k == 0), stop=(k == DT - 1))
                for k in range(DT):
                    nc.tensor.matmul(ssq, r(ones), r(sq[:, k, :]),
                                     start=(k == 0), stop=(k == DT - 1))
                m2 = spool.tile([1, 512], f32, tag="m2")
                nc.scalar.square(out=m2, in_=mu[0:1, :])
                vD2 = spool.tile([1, 512], f32, tag="vD2")
                nc.vector.scalar_tensor_tensor(
                    out=vD2, in0=ssq[0:1, :], scalar=float(D), in1=m2,
                    op0=OP.mult, op1=OP.subtract)
                # t = sqrt(vD2 + D^2*eps);  rD = 1/t = rstd/D
                tsq = spool.tile([1, 512], f32r, tag="tsq")
                nc.scalar.activation(out=tsq, in_=vD2, func=AF.Sqrt,
                                     bias=epsc, scale=1.0)
                # replicate mu and tsq across partitions via PE (ones-row
                # outer products into psum); rD_b = 1/tsq via fast approx
                mu_sb = spool.tile([1, 512], f32r, tag="mu_sb")
                nc.scalar.copy(out=mu_sb, in_=mu[0:1, :])
                mu_b = psum_st.tile([PART, 512], f32, tag="mu_b", bufs=2)
                nc.tensor.matmul(mu_b, r(ones_row), r(mu_sb),
                                 start=True, stop=True)
                ts_b = psum_st.tile([PART, 512], f32, tag="ts_b", bufs=2)
                nc.tensor.matmul(ts_b, r(ones_row), r(tsq),
                                 start=True, stop=True)
                rD_b = bpool.tile([PART, 512], f32, tag="rD_b")
                nc.vector.reciprocal_approx_fast(out=rD_b, in_=ts_b)
                for k in range(DT):
                    t1 = spool.tile([PART, 512], f32, tag="t1")
                    nc.vector.scalar_tensor_tensor(
                        out=t1, in0=src[:, k, cs], scalar=float(D), in1=mu_b,
                        op0=OP.mult, op1=OP.subtract)
                    if ln_degenerate:
                        nc.vector.tensor_mul(dst[:, k, cs], t1, rD_b)
                    else:
                        t2 = spool.tile([PART, 512], f32, tag="t2")
                        nc.vector.tensor_mul(t2, t1, rD_b)
                        nc.vector.tensor_scalar(
                            out=dst[:, k, cs], in0=t2,
                            scalar1=cur["lnw"][:, which, li:li + 1],
                            scalar2=cur["lnb"][:, which, li:li + 1],
                            op0=OP.mult, op1=OP.add)

        for li, (strm_name, vals_name, strict, ffn, fi) in enumerate(LAYERS):
            strm = yT if strm_name == "y" else xT
            valsT = yT if vals_name == "y" else strm
            lname = f"l{li}"
            with ExitStack() as lc:
                GT = gt_pool.tile([PART, DT, L], f32r, tag="GT",
                                  name=f"GT_{lname}")

                wk_sb = wkpool.tile([PART, DT, D], f32r, tag="wk", name=f"wk_{lname}")
                wv_sb = wpool.tile([PART, DT, D], f32r, tag="wv", name=f"wv_{lname}")
                wo_sb = wpool.tile([PART, DT, D], f32r, tag="wo", name=f"wo_{lname}")
                bk_sb = wpool.tile([PART, DT], f32, tag="bk", name=f"bk_{lname}")
                bo_sb = wpool.tile([PART, DT], f32, tag="bo", name=f"bo_{lname}")
                bv_b = None
                if not bv_zero:
                    bv_b = wpool.tile([PART, D], f32, tag="bv_b", name=f"bvb_{lname}")
                nc.sync.dma_start(out=wk_sb, in_=wk_d[li].rearrange("(k p) n -> p k n", p=PART))
                nc.sync.dma_start(out=wv_sb, in_=wv_d[li].rearrange("(k p) n -> p k n", p=PART))
                nc.sync.dma_start(out=wo_sb, in_=wo_d[li].rearrange("(k p) n -> p k n", p=PART))
                nc.sync.dma_start(out=bk_sb, in_=bk_d[li].rearrange("(k p) -> p k", p=PART))
                nc.sync.dma_start(out=bo_sb, in_=bo_d[li].rearrange("(k p) -> p k", p=PART))
                if not bv_zero:
                    nc.sync.dma_start(
                        out=bv_b,
                        in_=bv_d[li].unsqueeze(0).to_broadcast([PART, D]))
                if not ln_degenerate:
                    lnw_sb = wpool.tile([PART, 2, DT], f32, tag="lnw", name=f"lnw_{lname}")
                    lnb_sb = wpool.tile([PART, 2, DT], f32, tag="lnb", name=f"lnb_{lname}")
                    nc.sync.dma_start(out=lnw_sb, in_=lnw_d[li].rearrange("i (k p) -> p i k", p=PART))
                    nc.sync.dma_start(out=lnb_sb, in_=lnb_d[li].rearrange("i (k p) -> p i k", p=PART))
                    cur["lnw"], cur["lnb"] = lnw_sb, lnb_sb

                # ---- attention ----
                with ExitStack() as ac:
                    psum_s = ac.enter_context(
                        tc.tile_pool(name=f"ps_{lname}", bufs=3, space="PSUM"))
                    psum_pv = ac.enter_context(
                        tc.tile_pool(name=f"ppv_{lname}", bufs=3, space="PSUM"))
                    epool = ac.enter_context(tc.tile_pool(name=f"ep_{lname}", bufs=2))
                    vpool = ac.enter_context(tc.tile_pool(name=f"vp_{lname}", bufs=1))
                    cpool = ac.enter_context(tc.tile_pool(name=f"cp_{lname}", bufs=1))
                    rpool = ac.enter_context(tc.tile_pool(name=f"rp_{lname}", bufs=2))

                    # QK projection: GT = Wk^T @ strm + bk  (transposed out)
                    # weight-stationary: both L-chunks consume one LDWEIGHTS
                    for c in range(LC):
                        for m in range(DT):
                            pp = psum_proj.tile([PART, 512], f32, tag="proj",
                                                name=f"pp{c}_{m}")
                            for k in range(DT):
                                nc.tensor.matmul(
                                    pp, r(wk_sb[:, k, m * PART:(m + 1) * PART]),
                                    r(strm[:, k, c * 512:(c + 1) * 512]),
                                    start=(k == 0), stop=(k == DT - 1))
                            nc.vector.tensor_scalar(
                                out=GT[:, m, c * 512:(c + 1) * 512], in0=pp,
                                scalar1=bk_sb[:, m:m + 1], scalar2=None,
                                op0=OP.add)

                    # V projection (natural layout + ones col per head):
                    # V1[:, lb, h, 0:64] = V rows of block lb, head h; col 64 = 1
                    V1 = vpool.tile([PART, NB, H, 65], f32r, name=f"V1_{lname}")
                    for lb in range(NB):
                        pp = psum_proj.tile([PART, 512], f32, tag="proj")
                        for k in range(DT):
                            nc.tensor.matmul(
                                pp, r(valsT[:, k, lb * PART:(lb + 1) * PART]),
                                r(wv_sb[:, k, :]),
                                start=(k == 0), stop=(k == DT - 1))
                        if bv_zero:
                            nc.any.tensor_copy(
                                out=V1[:, lb, :, 0:64],
                                in_=pp.rearrange("p (h v) -> p h v", h=H))
                        else:
                            nc.any.tensor_tensor(
                                out=V1[:, lb, :, 0:64],
                                in0=pp.rearrange("p (h v) -> p h v", h=H),
                                in1=bv_b.rearrange("p (h v) -> p h v", h=H),
                                op=OP.add)
                    nc.vector.tensor_copy(
                        V1[:, :, :, 64:65],
                        ones_f.unsqueeze(1).unsqueeze(1).to_broadcast(
                            [PART, NB, H, 1]))

                    CtxT = cpool.tile([PART, DT, L], f32r, name=f"Ctx_{lname}")
                    mask_i = 1 if strict else 0

                    for h in range(H):
                        t, rb = h // 2, 64 * (h % 2)
                        qrow = slice(rb, rb + 64)
                        E = epool.tile([PART, NB, L], f32r, tag="E")
                        for j in range(NB):
                            for c in range(LC):
                                if j > 4 * c + 3:
                                    continue
                                off = max(0, j * PART - c * 512)
                                q0 = c * 512 + off
                                sp = psum_s.tile([PART, 512], f32, tag="s")
                                nc.tensor.matmul(
                                    sp[:, off:],
                                    r(GT[qrow, t, j * PART:(j + 1) * PART]),
                                    r(GT[qrow, t, q0:(c + 1) * 512]),
                                    start=True, stop=True)
                                nc.scalar.activation(
                                    out=E[:, j, q0:(c + 1) * 512], in_=sp[:, off:],
                                    func=AF.Exp, scale=0.125)
                        # mask the 8 diagonal blocks in two strided ops;
                        # diag of tile j sits at flat col 1152*j (j*1024+j*128)
                        mbc = masks[:, mask_i, :].unsqueeze(1).to_broadcast(
                            [PART, 4, PART])
                        for g in range(2):
                            dbase = E[:, 4 * g, 4 * g * PART:]
                            diag = bass.AP(
                                tensor=dbase.tensor, offset=dbase.offset,
                                ap=[list(dbase.ap[0]), [1152, 4], [1, PART]])
                            nc.gpsimd.tensor_mul(diag, diag, mbc)

                        pvs = [psum_pv.tile([PART, 512], f32, tag="pv",
                                             name=f"pv{c}")
                               for c in range(LC)]
                        for j in range(NB):
                            for c in range(LC):
                                if j > 4 * c + 3:
                                    continue
                                off = max(0, j * PART - c * 512)
                                q0 = c * 512 + off
                                nc.tensor.matmul(
                                    pvs[c][0:65, off:],
                                    r(V1[:, j, h, :]),
                                    r(E[:, j, q0:(c + 1) * 512]),
                                    start=(j == 0), stop=(j == 4 * c + 3))
                        for c in range(LC):
                            pvp = pvs[c]
                            S1 = rpool.tile([PART, 512], f32, tag="S1")
                            nc.scalar.copy(out=S1[64:65, :], in_=pvp[64:65, :])
                            Rb = rpool.tile([PART, 512], f32, tag="Rb")
                            dram_bcast(Rb[0:64, :], S1[64:65, :], 64)
                            nc.vector.reciprocal_approx_fast(
                                out=Rb[0:64, :], in_=Rb[0:64, :])
                            if strict and c == 0:
                                nc.vector.memset(Rb[0:64, 0:1], 0.0)
                            cs512 = slice(c * 512, (c + 1) * 512)
                            if h % 2 == 0:
                                nc.vector.tensor_mul(
                                    CtxT[0:64, t, cs512],
                                    pvp[0:64, :], Rb[0:64, :])
                            else:
                                ctmp = rpool.tile([PART, 512], f32r, tag="ctmp")
                                nc.vector.tensor_mul(
                                    ctmp[0:64, :], pvp[0:64, :], Rb[0:64, :])
                                nc.sync.dma_start(
                                    out=CtxT[64:128, t, cs512],
                                    in_=ctmp[0:64, :])

                    # O projection + bias + residual -> s in GT (scratch)
                    for m in range(DT):
                        pps = [psum_proj.tile([PART, 512], f32, tag="proj",
                                               name=f"pp{c}")
                               for c in range(LC)]
                        for k in range(DT):
                            for c in range(LC):
                                nc.tensor.matmul(
                                    pps[c], r(wo_sb[:, k, m * PART:(m + 1) * PART]),
                                    r(CtxT[:, k, c * 512:(c + 1) * 512]),
                                    start=(k == 0), stop=(k == DT - 1))
                        for c in range(LC):
                            cs = slice(c * 512, (c + 1) * 512)
                            nc.vector.scalar_tensor_tensor(
                                out=GT[:, m, cs], in0=pps[c],
                                scalar=bo_sb[:, m:m + 1], in1=strm[:, m, cs],
                                op0=OP.add, op1=OP.add)

                # ---- LN1: strm = LN(GT) ----
                with ExitStack() as nlc:
                    layer_norm(nlc, li, 0, GT, strm, f"{lname}a")

                # ---- FFN ----
                if ffn:
                    with ExitStack() as fc:
                        w1p = fc.enter_context(tc.tile_pool(name=f"w1_{lname}", bufs=3))
                        w2p = fc.enter_context(tc.tile_pool(name=f"w2_{lname}", bufs=1))
                        fp = fc.enter_context(tc.tile_pool(name=f"f_{lname}", bufs=1))
                        b1p = fc.enter_context(tc.tile_pool(name=f"b1_{lname}", bufs=1))
                        psum_o2 = fc.enter_context(
                            tc.tile_pool(name=f"po2_{lname}", bufs=2, space="PSUM"))

                        w2_sb = w2p.tile([PART, FT, D], f32r, tag="w2")
                        nc.sync.dma_start(
                            out=w2_sb, in_=w2_d[fi].rearrange("(k p) n -> p k n", p=PART))
                        b1_sb = b1p.tile([PART, FT], f32, tag="b1s")
                        b2_sb = b1p.tile([PART, DT], f32, tag="b2s")
                        nc.sync.dma_start(out=b1_sb, in_=b1_d[fi].rearrange("(k p) -> p k", p=PART))
                        nc.sync.dma_start(out=b2_sb, in_=b2_d[fi].rearrange("(k p) -> p k", p=PART))

                        F = fp.tile([PART, FT, L], f32r, name=f"F_{lname}")
                        for k in range(FT):
                            w1t = w1p.tile([PART, DT, PART], f32r, tag="w1")
                            nc.sync.dma_start(
                                out=w1t,
                                in_=w1_d[fi, :, k * PART:(k + 1) * PART]
                                .rearrange("(d p) n -> p d n", p=PART))
                            pps = [psum_proj.tile([PART, 512], f32,
                                                  tag="proj", name=f"fp{c}")
                                   for c in range(LC)]
                            for d in range(DT):
                                for c in range(LC):
                                    nc.tensor.matmul(
                                        pps[c], r(w1t[:, d, :]),
                                        r(strm[:, d, c * 512:(c + 1) * 512]),
                                        start=(d == 0), stop=(d == DT - 1))
                            for c in range(LC):
                                nc.vector.tensor_scalar(
                                    out=F[:, k, c * 512:(c + 1) * 512],
                                    in0=pps[c],
                                    scalar1=b1_sb[:, k:k + 1], scalar2=0.0,
                                    op0=OP.add, op1=OP.max)
                        for m in range(DT):
                            op2s = [psum_o2.tile([PART, 512], f32, tag="o2",
                                                 name=f"o2{c}")
                                    for c in range(LC)]
                            for k in range(FT):
                                for c in range(LC):
                                    nc.tensor.matmul(
                                        op2s[c],
                                        r(w2_sb[:, k, m * PART:(m + 1) * PART]),
                                        r(F[:, k, c * 512:(c + 1) * 512]),
                                        start=(k == 0), stop=(k == FT - 1))
                            for c in range(LC):
                                cs = slice(c * 512, (c + 1) * 512)
                                nc.vector.scalar_tensor_tensor(
                                    out=GT[:, m, cs], in0=op2s[c],
                                    scalar=b2_sb[:, m:m + 1], in1=strm[:, m, cs],
                                    op0=OP.add, op1=OP.add)
                    with ExitStack() as nlc:
                        layer_norm(nlc, li, 1, GT, strm, f"{lname}b")

        nc.sync.dma_start(out=out_d.rearrange("(t p) l -> p t l", p=PART), in_=xT)


_LDW_PATCHED = [False]


def _maybe_patch_ldw_opt():
    """Enable walrus LDWEIGHTS dedup (--enable-ldw-opt) so repeated
    stationary operands aren't reloaded; gated by KERNEL_LDWOPT env."""
    import os
    if _LDW_PATCHED[0] or os.environ.get("KERNEL_LDWOPT", "1") != "1":
        return
    import concourse.bass_utils as bu
    orig = bu.run_command

    def patched(cmd, **kw):
        cmd = ["--enable-ldw-opt=true" if c == "--enable-ldw-opt=false" else c
               for c in cmd]
        return orig(cmd, **kw)

    bu.run_command = patched
    _LDW_PATCHED[0] = True


def _build(ln_degenerate, bv_zero):
    import concourse.bacc as bacc

    _maybe_patch_ldw_opt()
    key = ("nc", ln_degenerate, bv_zero)
    if key in _CACHE:
        return _CACHE[key]
    nc = bacc.Bacc("TRN2", target_bir_lowering=False, debug=False,
                   num_devices=NCORES)
    _emit(nc, ln_degenerate, bv_zero)
    nc.compile()
    _CACHE[key] = nc
    return nc


def _prepare(inputs):
    q = np.ascontiguousarray(np.asarray(inputs["q_embed_data"], dtype=np.float32))
    qa = np.ascontiguousarray(np.asarray(inputs["qa_embed_data"], dtype=np.float32))
    p = inputs["params"]

    def g(block, name):
        return np.asarray(p[block][name], dtype=np.float32)

    def stack6(name):
        return np.ascontiguousarray(
            np.concatenate([g("b1", name), g("b2", name)], axis=0))

    wk, wv, wo = stack6("Wk"), stack6("Wv"), stack6("Wo")
    bk, bv, bo = stack6("bk"), stack6("bv"), stack6("bo")
    lnw = np.ascontiguousarray(np.stack([stack6("ln1_w"), stack6("ln2_w")], axis=1))
    lnb = np.ascontiguousarray(np.stack([stack6("ln1_b"), stack6("ln2_b")], axis=1))
    ffn_layers = [gl for gl, cfg in enumerate(LAYERS) if cfg[3]]
    w1 = np.ascontiguousarray(stack6("W1")[ffn_layers])
    b1 = np.ascontiguousarray(stack6("b1")[ffn_layers])
    w2 = np.ascontiguousarray(stack6("W2")[ffn_layers])
    b2 = np.ascontiguousarray(stack6("b2")[ffn_layers])

    ln_degenerate = bool(np.all(lnw == 1.0) and np.all(lnb == 0.0))
    bv_zero = bool(np.all(bv == 0.0))

    shared = dict(wk=wk, wv=wv, wo=wo, bk=bk, bv=bv, bo=bo,
                  lnw=lnw, lnb=lnb, w1=w1, b1=b1, w2=w2, b2=b2)
    in_maps = []
    for b in range(B):
        m = dict(shared)
        m["xT"] = np.ascontiguousarray(q[b].T)
        m["yT"] = np.ascontiguousarray(qa[b].T)
        in_maps.append(m)
    return in_maps, ln_degenerate, bv_zero


def run(inputs, trace=False):
    from concourse.bass_utils import run_bass_kernel_spmd

    in_maps, ln_degenerate, bv_zero = _prepare(inputs)
    nc = _build(ln_degenerate, bv_zero)
    res = run_bass_kernel_spmd(nc, in_maps, core_ids=list(range(NCORES)),
                               trace=trace)
    out = np.stack([res.results[b]["outT"].T for b in range(B)])
    return out, res


def kernel(**inputs):
    out, _ = run(inputs, trace=False)
    return out
